# revision 1
# baseline (speedup 1.0000x reference)
"""Trainium2 Bass kernel: GQA attention layer with RoPE + int8 quant-dequant KV.

Tensor-parallel over heads across 8 NeuronCores: core c owns q-heads
[4c, 4c+4) and kv-head c.  Each core computes its partial output
y_c = attn_out_local @ wo_local.T; the host sums the 8 partials.

Per-core dataflow (all "T" tensors are [feature, token] with feature on
SBUF partitions):
  hsT --matmul(f32r)--> qT/kT/vT --RoPE(perm-matmul + DVE)-->
  --int8 quant-dequant (gpsimd absmax + magic-round on DVE)-->
  scores sT[k,q] = kT.T @ qT --exp(ACT)--> masked --> AV + ones-sum (PE)
  --> normalize --> aT(bf16) --wo matmul(bf16)--> y partial
"""
import math
import numpy as np
from contextlib import ExitStack

import concourse.bass as bass
import concourse.bacc as bacc
import concourse.mybir as mybir
import concourse.tile as tile
from concourse.bass_utils import run_bass_kernel_spmd
from concourse.masks import make_identity

F32 = mybir.dt.float32
F32R = mybir.dt.float32r
BF16 = mybir.dt.bfloat16
AF = mybir.ActivationFunctionType
ALU = mybir.AluOpType
AX = mybir.AxisListType

MAGIC = 1.5 * 2.0**23  # fp32 RNE integer-rounding magic constant
NCORES = 8


def build_nc(S=2048, D=4096, HL=4, QT=512, MMDT=BF16):
    """Build the per-core Bass graph. HL = local q heads (1 local kv head)."""
    DT = D // 128    # contraction tiles for projections
    NQ = S // QT     # query tiles
    DB = QT // 128   # 128-blocks per query tile
    KB = S // 128    # total k blocks
    NDC = D // 512   # wo output column tiles

    nc = bacc.Bacc("TRN2")
    hsT = nc.declare_dram_parameter("hsT", [D, S], MMDT, isOutput=False)
    wqT = nc.declare_dram_parameter("wqT", [D, HL * 128], MMDT, isOutput=False)
    wkT = nc.declare_dram_parameter("wkT", [D, 128], MMDT, isOutput=False)
    wvT = nc.declare_dram_parameter("wvT", [D, 128], MMDT, isOutput=False)
    woT = nc.declare_dram_parameter("woT", [HL * 128, D], BF16, isOutput=False)
    cosT = nc.declare_dram_parameter("cosT", [128, S], F32, isOutput=False)
    sinT = nc.declare_dram_parameter("sinT", [128, S], F32, isOutput=False)
    mks = nc.declare_dram_parameter("mks", [DB, 128, QT], F32, isOutput=False)
    rotT = nc.declare_dram_parameter("rotT", [128, 128], MMDT, isOutput=False)
    y = nc.declare_dram_parameter("y", [S, D], F32, isOutput=True)

    with tile.TileContext(nc) as tc, ExitStack() as ctx:
        const = ctx.enter_context(tc.tile_pool(name="const", bufs=1))
        persist = ctx.enter_context(tc.tile_pool(name="persist", bufs=1))
        hs_pool = ctx.enter_context(tc.tile_pool(name="hs", bufs=4))
        wq_pool = ctx.enter_context(tc.tile_pool(name="wqp", bufs=4))
        wkv_pool = ctx.enter_context(tc.tile_pool(name="wkvp", bufs=4))
        work = ctx.enter_context(tc.tile_pool(name="work", bufs=6))
        work128 = ctx.enter_context(tc.tile_pool(name="work128", bufs=6))
        expp = ctx.enter_context(tc.tile_pool(name="expp", bufs=4))
        qpool = ctx.enter_context(tc.tile_pool(name="qpool", bufs=2 * HL))
        apool = ctx.enter_context(tc.tile_pool(name="apool", bufs=2 * HL))
        ypool = ctx.enter_context(tc.tile_pool(name="ypool", bufs=4))
        rows = ctx.enter_context(tc.tile_pool(name="rows", bufs=8))
        pbig = ctx.enter_context(tc.tile_pool(name="pbig", bufs=7, space="PSUM"))
        psum1 = ctx.enter_context(tc.tile_pool(name="psum1", bufs=1, space="PSUM"))
        drampool = ctx.enter_context(tc.tile_pool(name="drampool", bufs=4, space="DRAM"))

        # ---- constants ----
        cos_sb = const.tile([128, S], F32, name="cos", tag="cos")
        nc.sync.dma_start(out=cos_sb[:], in_=cosT[:])
        sin_sb = const.tile([128, S], F32, name="sin", tag="sin")
        nc.sync.dma_start(out=sin_sb[:], in_=sinT[:])
        mks_sb = []
        for r in range(DB):
            m = const.tile([128, QT], F32, name=f"mk{r}", tag=f"mk{r}")
            nc.sync.dma_start(out=m[:], in_=mks[r, :, :])
            mks_sb.append(m)
        rot_sb = const.tile([128, 128], MMDT, name="rot", tag="rot")
        nc.sync.dma_start(out=rot_sb[:], in_=rotT[:])
        ident = const.tile([128, 128], F32, name="ident", tag="ident")
        make_identity(nc, ident[:])
        ones_col = const.tile([128, 1], MMDT, name="onec", tag="onec")
        nc.vector.memset(ones_col[:], 1.0)
        ones_row = const.tile([1, 128], MMDT, name="oner", tag="oner")
        nc.vector.memset(ones_row[:], 1.0)
        zbias = const.tile([128, 1], F32, name="zbias", tag="zbias")
        nc.vector.memset(zbias[:], 0.0)

        kT_all = persist.tile([128, S], MMDT, name="kT", tag="kT")
        v_nat = persist.tile([128, KB, 128], MMDT, name="vnat", tag="vnat")
        woT_sb = []
        for hb in range(HL):
            w = persist.tile([128, D], BF16, name=f"wo{hb}", tag=f"wo{hb}")
            nc.sync.dma_start(out=w[:], in_=woT[hb * 128:(hb + 1) * 128, :])
            woT_sb.append(w)

        def qd_nat_block(x_ap, out_ap):
            """int8 quant-dequant of one [tok(part), dh(free)] 128x128 block.

            absmax over the free (dh) axis per token, symmetric 127-step
            grid, round-to-nearest-even via the fp32 magic trick.
            """
            amax = rows.tile([128, 1], F32, name="row", tag="row")
            nc.vector.tensor_reduce(out=amax[:], in_=x_ap, axis=AX.X,
                                    op=ALU.max, apply_absolute_value=True)
            scl = rows.tile([128, 1], F32, name="row", tag="row")
            nc.vector.tensor_scalar(out=scl[:], in0=amax[:],
                                    scalar1=1.0 / 127.0, scalar2=1e-8,
                                    op0=ALU.mult, op1=ALU.max)
            inv = rows.tile([128, 1], F32, name="row", tag="row")
            nc.vector.reciprocal(inv[:], scl[:])
            xs = work128.tile([128, 128], F32, name="w128", tag="w128")
            nc.vector.tensor_scalar(out=xs[:], in0=x_ap, scalar1=inv[:],
                                    scalar2=None, op0=ALU.mult)
            nc.vector.tensor_scalar(out=xs[:], in0=xs[:], scalar1=MAGIC,
                                    scalar2=MAGIC, op0=ALU.add,
                                    op1=ALU.subtract)
            nc.vector.tensor_scalar(out=out_ap, in0=xs[:], scalar1=scl[:],
                                    scalar2=None, op0=ALU.mult)

        def rope(psum_in, cos_sl, sin_sl, out_ap):
            """RoPE in [feat, tok] layout; rotate-half via permutation matmul."""
            raw = work.tile([128, QT], MMDT, name="rawmm", tag="rawmm")
            nc.vector.tensor_copy(raw[:], psum_in[:])
            rot_ps = pbig.tile([128, QT], F32, name="big", tag="big")
            nc.tensor.matmul(rot_ps[:], rot_sb[:],
                             raw[:], start=True, stop=True)
            tmp = work.tile([128, QT], F32, name="work", tag="work")
            nc.vector.tensor_tensor(out=tmp[:], in0=raw[:], in1=cos_sl,
                                    op=ALU.mult)
            t2 = work.tile([128, QT], F32, name="work", tag="work")
            nc.vector.tensor_tensor(out=t2[:], in0=rot_ps[:], in1=sin_sl,
                                    op=ALU.mult)
            nc.vector.tensor_tensor(out=out_ap, in0=tmp[:], in1=t2[:],
                                    op=ALU.add)

        for I in range(NQ):
            qsl = slice(I * QT, (I + 1) * QT)
            cos_sl = cos_sb[:, qsl]
            sin_sl = sin_sb[:, qsl]

            # ---- q/k/v projections for this token tile ----
            pq = [pbig.tile([128, QT], F32, name="big", tag="big") for _ in range(HL)]
            pk = pbig.tile([128, QT], F32, name="big", tag="big")
            pv = pbig.tile([128, QT], F32, name="big", tag="big")
            for d in range(DT):
                dsl = slice(d * 128, (d + 1) * 128)
                hs_t = hs_pool.tile([128, QT], MMDT, name="hs", tag="hs")
                nc.sync.dma_start(out=hs_t[:], in_=hsT[dsl, qsl])
                wq_t = wq_pool.tile([128, HL * 128], MMDT, name="wq", tag="wq")
                nc.sync.dma_start(out=wq_t[:], in_=wqT[dsl, :])
                wk_t = wkv_pool.tile([128, 128], MMDT, name="wkv", tag="wkv")
                nc.sync.dma_start(out=wk_t[:], in_=wkT[dsl, :])
                wv_t = wkv_pool.tile([128, 128], MMDT, name="wkv", tag="wkv")
                nc.sync.dma_start(out=wv_t[:], in_=wvT[dsl, :])
                first, last = d == 0, d == DT - 1
                for h in range(HL):
                    nc.tensor.matmul(pq[h][:],
                                     wq_t[:, h * 128:(h + 1) * 128],
                                     hs_t[:],
                                     start=first, stop=last)
                nc.tensor.matmul(pk[:], wk_t[:],
                                 hs_t[:], start=first, stop=last)
                nc.tensor.matmul(pv[:], wv_t[:],
                                 hs_t[:], start=first, stop=last)

            # ---- RoPE q ----
            qts = []
            for h in range(HL):
                qt_t = qpool.tile([128, QT], MMDT, name="qt", tag="qt")
                rope(pq[h], cos_sl, sin_sl, qt_t[:])
                qts.append(qt_t)

            # ---- RoPE k; per 128-block: transpose -> qd -> transpose back ----
            krope = work.tile([128, QT], F32, name="work", tag="work")
            rope(pk, cos_sl, sin_sl, krope[:])
            for t in range(DB):
                t_sl = slice(t * 128, (t + 1) * 128)
                tr_ps = pbig.tile([128, 128], F32, name="big", tag="big")
                nc.tensor.transpose(tr_ps[:], krope[:, t_sl], ident[:])
                k_nat = work128.tile([128, 128], F32, name="w128", tag="w128")
                nc.vector.tensor_copy(k_nat[:], tr_ps[:])
                kq_nat = work128.tile([128, 128], F32, name="w128", tag="w128")
                qd_nat_block(k_nat[:], kq_nat[:])
                tr2_ps = pbig.tile([128, 128], F32, name="big", tag="big")
                nc.tensor.transpose(tr2_ps[:], kq_nat[:], ident[:])
                nc.vector.tensor_copy(kT_all[:, I * QT + t * 128:
                                              I * QT + (t + 1) * 128], tr2_ps[:])

            # ---- v: transpose to natural [tok, dh] blocks, then qd ----
            vraw = work.tile([128, QT], F32, name="work", tag="work")
            nc.vector.tensor_copy(vraw[:], pv[:])
            for t in range(DB):
                t_sl = slice(t * 128, (t + 1) * 128)
                tr_ps = pbig.tile([128, 128], F32, name="big", tag="big")
                nc.tensor.transpose(tr_ps[:], vraw[:, t_sl], ident[:])
                v_nat_raw = work128.tile([128, 128], F32, name="w128", tag="w128")
                nc.vector.tensor_copy(v_nat_raw[:], tr_ps[:])
                qd_nat_block(v_nat_raw[:], v_nat[:, I * DB + t, :])

            # ---- attention (causal, unnormalized exp + ones-sum) ----
            ats = []
            nkb = (I + 1) * DB
            for h in range(HL):
                out_ps = pbig.tile([128, QT], F32, name="big", tag="big")
                sum_ps = psum1.tile([1, QT], F32, name="sum", tag="sum")
                for j in range(nkb):
                    s_ps = pbig.tile([128, QT], F32, name="big", tag="big")
                    nc.tensor.matmul(s_ps[:],
                                     kT_all[:, j * 128:(j + 1) * 128],
                                     qts[h][:],
                                     start=True, stop=True)
                    e_sb = expp.tile([128, QT], MMDT, name="exp", tag="exp")
                    nc.scalar.activation(e_sb[:], s_ps[:], AF.Exp,
                                         bias=zbias[:],
                                         scale=1.0 / math.sqrt(128.0))
                    r = j - I * DB
                    if r >= 0:
                        nc.vector.tensor_tensor(out=e_sb[:], in0=e_sb[:],
                                                in1=mks_sb[r][:], op=ALU.mult)
                    first, last = j == 0, j == nkb - 1
                    nc.tensor.matmul(out_ps[:], v_nat[:, j, :],
                                     e_sb[:],
                                     start=first, stop=last)
                    nc.tensor.matmul(sum_ps[:], ones_col[:],
                                     e_sb[:],
                                     start=first, stop=last)
                rec = rows.tile([1, QT], F32, name="rec", tag="rec")
                nc.vector.reciprocal(rec[:], sum_ps[:])
                rec_d = drampool.tile([1, QT], F32, name="recd", tag="recd")
                nc.sync.dma_start(out=rec_d[:], in_=rec[:])
                brec = work.tile([128, QT], F32, name="work", tag="work")
                rec_bcast = bass.AP(
                    tensor=rec_d.tensor, offset=rec_d.offset,
                    ap=[[0, 128]] + list(rec_d.ap[1:]))
                nc.sync.dma_start(out=brec[:], in_=rec_bcast)
                a_t = apool.tile([128, QT], BF16, name="at", tag="at")
                nc.vector.tensor_tensor(out=a_t[:], in0=out_ps[:],
                                        in1=brec[:], op=ALU.mult)
                ats.append(a_t)

            # ---- wo partial: y[tok, dout] += aT.T @ woT ----
            for t in range(DB):
                t_sl = slice(t * 128, (t + 1) * 128)
                for dc in range(NDC):
                    y_ps = pbig.tile([128, 512], F32, name="big", tag="big")
                    for hb in range(HL):
                        nc.tensor.matmul(y_ps[:], ats[hb][:, t_sl],
                                         woT_sb[hb][:, dc * 512:(dc + 1) * 512],
                                         start=(hb == 0), stop=(hb == HL - 1))
                    y_sb = ypool.tile([128, 512], F32, name="y", tag="y")
                    nc.vector.tensor_copy(y_sb[:], y_ps[:])
                    nc.sync.dma_start(
                        out=y[I * QT + t * 128:I * QT + (t + 1) * 128,
                              dc * 512:(dc + 1) * 512],
                        in_=y_sb[:])
    nc.compile()
    return nc


def host_inputs(hidden_states, wq, wk, wv, wo, position_ids,
                S=2048, D=4096, HL=4, QT=512, ncores=NCORES, mmdt="bf16"):
    """Shard + preprocess inputs -> per-core in_maps."""
    import ml_dtypes
    cast = ((lambda a: np.ascontiguousarray(a).astype(ml_dtypes.bfloat16))
            if mmdt == "bf16" else (lambda a: np.ascontiguousarray(a)))
    DB = QT // 128
    hs = np.asarray(hidden_states, np.float32)[0]
    hsT = np.ascontiguousarray(hs.T)  # [D, S]

    pos = np.asarray(position_ids)[0].astype(np.float32)
    inv_freq = (1.0 / (10000.0 ** (np.arange(0, 128, 2, dtype=np.float32) / 128.0)))
    freqs = pos[:, None] * inv_freq[None, :]          # [S, 64]
    emb = np.concatenate([freqs, freqs], axis=1)      # [S, 128]
    cosT = np.ascontiguousarray(np.cos(emb).T).astype(np.float32)
    sinT = np.ascontiguousarray(np.sin(emb).T).astype(np.float32)

    kk = np.arange(128)[:, None]
    qq = np.arange(QT)[None, :]
    mks = np.stack([(kk + 128 * r <= qq) for r in range(DB)]).astype(np.float32)

    rotT = np.zeros((128, 128), np.float32)
    idx = np.arange(64)
    rotT[idx, idx + 64] = 1.0
    rotT[idx + 64, idx] = -1.0
    rotT = cast(rotT)

    wq = np.asarray(wq, np.float32)
    wk = np.asarray(wk, np.float32)
    wv = np.asarray(wv, np.float32)
    wo = np.asarray(wo, np.float32)

    hsT = cast(hsT)
    in_maps = []
    qh = HL * 128
    for c in range(ncores):
        wqT_c = cast(wq[c * qh:(c + 1) * qh, :].T)
        wkT_c = cast(wk[c * 128:(c + 1) * 128, :].T)
        wvT_c = cast(wv[c * 128:(c + 1) * 128, :].T)
        woT_c = np.ascontiguousarray(wo[:, c * qh:(c + 1) * qh].T).astype(
            ml_dtypes.bfloat16)
        in_maps.append({
            "hsT": hsT, "wqT": wqT_c, "wkT": wkT_c, "wvT": wvT_c,
            "woT": woT_c, "cosT": cosT, "sinT": sinT, "mks": mks,
            "rotT": rotT,
        })
    return in_maps


_NC_CACHE = {}
COMPUTE = "bf16"  # "bf16" or "f32r"


def kernel(hidden_states, wq, wk, wv, wo, position_ids):
    B, S, D = hidden_states.shape
    in_maps = host_inputs(hidden_states, wq, wk, wv, wo, position_ids,
                          S=S, D=D, mmdt=COMPUTE)
    key = (S, D, COMPUTE)
    if key not in _NC_CACHE:
        _NC_CACHE[key] = build_nc(S=S, D=D,
                                  MMDT=BF16 if COMPUTE == "bf16" else F32R)
    nc = _NC_CACHE[key]
    res = run_bass_kernel_spmd(nc, in_maps, core_ids=list(range(NCORES)),
                               trace=False)
    y = np.zeros((S, D), np.float64)
    for c in range(NCORES):
        y += res.results[c]["y"].astype(np.float64)
    return y.astype(np.float32)[None]

